# revision 7
# baseline (speedup 1.0000x reference)
"""Trainium2 Bass kernel: Brevitas-style int4 fake-quant Conv2d (3x3, pad 1).

reference:
    wq = fake_quant_per_channel(w)          # per-O-channel int4 scale
    out = conv2d(x, wq, NCHW/OIHW, pad 1)

Strategy (v2, 1D Winograd F(4,3) along H):
  * Host: fake-quant w -> wq (f32), then fold the Winograd weight
    transform G (and the per-channel scale, already inside wq) into
    U[u, o, c, kw] = sum_r G[u, r] wq[o, c, r, kw], cast fp16.
  * Device (data-parallel, 4 images/core x 8 cores): for each (img, kt)
    build padded fp16 xp [128, 58, 58], then 6 Winograd input planes
    V_u[c, t, col] = sum_r B^T[u, r] xp[c, 4t+r, col] (t = 14 row tiles)
    via 14 DVE tensor/scalar_tensor_tensor ops (all +-{1,2,4,5} combos).
    The conv becomes, per (ot, chunk of 7 tiles): 6 accumulation groups
    M_u [128o, 7, 56] = sum_{kt, kw} U[u,kw,kt,ot].T @ V_u[kt][:, tiles,
    kw:kw+56] -- 36 matmuls of 392 cols vs 72 equivalent direct-conv
    matmuls: 2x fewer PE cycles than the direct implicit GEMM.
  * Output transform (A^T) during the drain, spread across engines:
      p=m1-m2 r=m1+m2 q=m3-m4 s=m3+m4 t0=m0+r w=m5+p   (any: DVE/ACT)
      o0=t0+s (gpsimd)  o1=2q+p o2=4s+r o3=8q+w (DVE STT)
    rows interleave 4t+j into the f32 out tile, DMA per (ot, chunk).
  * Accuracy: host-sim of this exact fp16 pipeline measures absmax rel
    err ~2.2e-3 vs the f32 reference (gate 2e-2).
"""

import os
import sys
from contextlib import ExitStack

for _p in ("/opt/trn_rl_repo", "/root/.axon_site/_ro/trn_rl_repo"):
    if os.path.isdir(_p) and _p not in sys.path:
        sys.path.insert(0, _p)

import numpy as np

import concourse.bass as bass  # noqa: F401
import concourse.mybir as mybir
import concourse.tile as tile
from concourse import bacc
from concourse.bass_utils import run_bass_kernel_spmd

F32 = mybir.dt.float32
FP16 = mybir.dt.float16

ALU = mybir.AluOpType

# Problem shapes (hardcoded per contract).
N, C, H, W = 32, 256, 56, 56
O, KH, KW = 256, 3, 3
CORES = 8
NPC = N // CORES  # images per core

QMAX = 7.0
SCALING_MIN_VAL = 2e-16

KT = C // 128
OT = O // 128
TW = H // 4            # 14 winograd row tiles
TR = TW // 2           # 7 tiles per chunk
NCH = 2                # chunks per (img, ot)
HP, WP = 58, 58        # used pad rows/cols
HPA = 60               # allocated rows (multiple of 4 for rearrange)
NSTRIP = 7             # x DMA strips of 8 rows
U_ORDER = (1, 2, 3, 4, 0, 5)

# F(4,3) transforms (points 0, +-1, +-2, inf)
G_MAT = np.array([
    [1 / 4, 0, 0],
    [-1 / 6, -1 / 6, -1 / 6],
    [-1 / 6, 1 / 6, -1 / 6],
    [1 / 24, 1 / 12, 1 / 6],
    [1 / 24, -1 / 12, 1 / 6],
    [0, 0, 1],
], dtype=np.float64)


def build_nc(npc=NPC, warmup_mms=85):
    nc = bacc.Bacc("TRN2", target_bir_lowering=False, debug=False)
    x_d = nc.dram_tensor("x", [npc, C, H, W], F32, kind="ExternalInput").ap()
    w_d = nc.dram_tensor("wu", [128, 6 * 3 * KT * OT * 128], FP16,
                         kind="ExternalInput").ap()
    out_d = nc.dram_tensor("out", [npc, O, H, W], F32,
                           kind="ExternalOutput").ap()

    with tile.TileContext(nc) as tc, ExitStack() as ctx:
        wpool = ctx.enter_context(tc.tile_pool(name="wpool", bufs=1))
        xspool = ctx.enter_context(tc.tile_pool(name="xspool", bufs=16))
        hpool = ctx.enter_context(tc.tile_pool(name="hpool", bufs=2))
        vpool = ctx.enter_context(tc.tile_pool(name="vpool", bufs=2))
        s16pool = ctx.enter_context(tc.tile_pool(name="s16pool", bufs=4))
        s32pool = ctx.enter_context(tc.tile_pool(name="s32pool", bufs=12))
        opool = ctx.enter_context(tc.tile_pool(name="opool", bufs=4))
        ppool = ctx.enter_context(tc.tile_pool(name="ppool", bufs=8,
                                               space="PSUM"))

        w_sb = wpool.tile([128, 6 * 3 * KT * OT * 128], FP16)
        nc.scalar.dma_start(w_sb[:, :], w_d[:, :])

        def wslice(u, kw, kt, ot):
            j = (((u * 3 + kw) * KT + kt) * OT + ot) * 128
            return w_sb[:, j:j + 128]

        if warmup_mms:
            wu = wpool.tile([128, 128], FP16)
            nc.vector.memset(wu[:, :], 0.0)
            wu_ps = ppool.tile([128, 512], F32, tag="ps", name="wu_ps")
            for _ in range(warmup_mms):
                nc.tensor.matmul(wu_ps[:, :128], wu[:, :], wu[:, :],
                                 start=True, stop=True)

        # per-(img, kt) xp and V tiles, created lazily
        xpv = {}   # (img, kt) -> [128, HPA, WP] fp16 view
        vv = {}    # (img, kt) -> [128, 6, TW, WP] fp16 view

        def emit_pad(img, kt):
            xp = hpool.tile([128, HPA * WP], FP16, tag=f"xp{kt}")
            v = xp[:, :].rearrange("p (r c) -> p r c", c=WP)
            xpv[(img, kt)] = v
            # zero borders: row 0 + (1,0); (56,57)+row 57; middle col pairs
            nc.vector.memset(xp[:, 0: WP + 1], 0.0)
            nc.vector.memset(xp[:, (HP - 1) * WP - 1: HP * WP], 0.0)
            nc.vector.memset(
                xp[:, 2 * WP - 1: 2 * WP - 1 + (HP - 3) * WP]
                .rearrange("p (a b) -> p a b", b=WP)[:, :, 0:2], 0.0)

        def emit_strip_triggers(img, kt, s0, s1):
            tiles = []
            for s in range(s0, s1):
                xs = xspool.tile([128, 8, W], F32, tag="xs")
                q = nc.sync if kt == 0 else nc.scalar
                q.dma_start(xs[:, :, :],
                            x_d[img, kt * 128:(kt + 1) * 128,
                                8 * s: 8 * s + 8, :])
                tiles.append(xs)
            return tiles

        def emit_converts(img, kt, s0, tiles):
            v = xpv[(img, kt)]
            for i, xs in enumerate(tiles):
                s = s0 + i
                nc.scalar.copy(v[:, 1 + 8 * s: 9 + 8 * s, 1:1 + W],
                               xs[:, :, :])

        def emit_V(img, kt, half):
            """Build V planes for tiles [half*TR, (half+1)*TR)."""
            if (img, kt) not in vv:
                vt = vpool.tile([128, 6 * TW * WP], FP16, tag=f"v{kt}")
                vv[(img, kt)] = vt[:, :].rearrange(
                    "p (u t c) -> p u t c", u=6, c=WP)
            v4 = vv[(img, kt)]
            ta, tb = half * TR, (half + 1) * TR
            n = tb - ta
            xp4 = xpv[(img, kt)].rearrange("p (t f) c -> p t f c", f=4)

            def R(i):
                q, rr = divmod(i, 4)
                return xp4[:, ta + q: tb + q, rr, :]

            def V(u):
                return v4[:, u, ta:tb, :]

            def s16():
                t = s16pool.tile([128, TR * WP], FP16, tag="s16")
                return t[:, :].rearrange("p (t c) -> p t c", c=WP)[:, :n, :]

            ve = nc.vector
            t1 = s16()
            ve.scalar_tensor_tensor(t1, R(2), -5.0, R(4), ALU.mult, ALU.add)
            ve.scalar_tensor_tensor(V(0), R(0), 4.0, t1, ALU.mult, ALU.add)
            a = s16()
            ve.tensor_add(a, R(1), R(2))
            b = s16()
            ve.tensor_add(b, R(3), R(4))
            ve.scalar_tensor_tensor(V(1), a, -4.0, b, ALU.mult, ALU.add)
            cc = s16()
            ve.tensor_sub(cc, R(1), R(2))
            d = s16()
            ve.tensor_sub(d, R(3), R(4))
            ve.scalar_tensor_tensor(V(2), cc, 4.0, d, ALU.mult, ALU.subtract)
            e = s16()
            ve.tensor_sub(e, R(3), R(1))
            f = s16()
            ve.tensor_sub(f, R(4), R(2))
            ve.scalar_tensor_tensor(V(3), e, 2.0, f, ALU.mult, ALU.add)
            ve.scalar_tensor_tensor(V(4), e, -2.0, f, ALU.mult, ALU.add)
            t5 = s16()
            ve.scalar_tensor_tensor(t5, R(3), -5.0, R(5), ALU.mult, ALU.add)
            ve.scalar_tensor_tensor(V(5), R(1), 4.0, t5, ALU.mult, ALU.add)

        def s32():
            t = s32pool.tile([128, TR * W], F32, tag="s32")
            return t[:, :].rearrange("p (t c) -> p t c", c=W)

        def emit_chunk(img, ci, ot, last=False):
            m = {}
            ob = opool.tile([128, 4 * TR * W], F32, tag="ob")
            ob4 = ob[:, :].rearrange("p (t f c) -> p t f c", f=4, c=W)
            sc = {}
            for gi, u in enumerate(U_ORDER):
                ps = ppool.tile([128, 512], F32, tag="ps", name=f"ps{u}")
                mv = ps[:, : TR * W].rearrange("p (t c) -> p t c", c=W)
                m[u] = mv
                idx = 0
                for kt in range(KT):
                    vsl = vv[(img, kt)][:, u, ci * TR:(ci + 1) * TR, :]
                    for kw in range(3):
                        nc.tensor.matmul(
                            mv[:, :, :], wslice(u, kw, kt, ot),
                            vsl[:, :, kw: kw + W],
                            start=(idx == 0), stop=(idx == 3 * KT - 1),
                        )
                        idx += 1
                # drains trail each completed accumulation group. TT is
                # DVE/Pool-only; Pool (gpsimd) can't read PSUM; TT may read
                # at most one PSUM operand. So: ACT copies even-u PSUM banks
                # to SBUF, DVE combines (1 PSUM max), gpsimd does SBUF-only.
                if u == 2:
                    c2 = s32()
                    nc.scalar.copy(c2, m[2])
                    pv, rv = s32(), s32()
                    nc.vector.tensor_sub(pv, m[1], c2)   # p = m1 - m2
                    nc.vector.tensor_add(rv, m[1], c2)   # r = m1 + m2
                    sc["p"], sc["r"] = pv, rv
                if u == 4:
                    c4 = s32()
                    nc.scalar.copy(c4, m[4])
                    qv, sv = s32(), s32()
                    nc.vector.tensor_sub(qv, m[3], c4)   # q = m3 - m4
                    nc.vector.tensor_add(sv, m[3], c4)   # s = m3 + m4
                    sc["q"], sc["s"] = qv, sv
                    q2 = s32()
                    nc.gpsimd.tensor_add(q2, qv, qv)
                    nc.gpsimd.tensor_add(ob4[:, :, 1, :], q2, sc["p"])
                    nc.vector.scalar_tensor_tensor(
                        ob4[:, :, 2, :], sv, 4.0, sc["r"],
                        ALU.mult, ALU.add)               # o2 = 4s + r
                if u == 0:
                    c0 = s32()
                    nc.scalar.copy(c0, m[0])
                    t0 = s32()
                    nc.gpsimd.tensor_add(t0, c0, sc["r"])  # t0 = m0 + r
                    nc.gpsimd.tensor_add(ob4[:, :, 0, :], t0, sc["s"])
            c5 = s32()
            nc.scalar.copy(c5, m[5])
            wv = s32()
            nc.gpsimd.tensor_add(wv, c5, sc["p"])        # w = m5 + p
            nc.vector.scalar_tensor_tensor(
                ob4[:, :, 3, :], sc["q"], 8.0, wv,
                ALU.mult, ALU.add)                       # o3 = 8q + w
            dst = out_d[img, ot * 128:(ot + 1) * 128,
                        4 * TR * ci: 4 * TR * (ci + 1), :]
            obv = ob[:, :].rearrange("p (r c) -> p r c", c=W)
            if last:
                hr = 2 * TR
                for a0, b0 in ((0, hr), (hr, 4 * TR)):
                    nc.sync.dma_start(dst[:, a0:b0, :], obv[:, a0:b0, :])
            else:
                nc.sync.dma_start(dst[:, :, :], obv[:, :, :])

        # ---------------- schedule ----------------
        # image 0: paced by DMA strips; V in (kt, half) quarters
        for kt in range(KT):
            emit_pad(0, kt)
        st0 = {kt: emit_strip_triggers(0, kt, 0, 4) for kt in range(KT)}
        for kt in range(KT):
            emit_converts(0, kt, 0, st0[kt])
        for kt in range(KT):
            emit_V(0, kt, 0)
        st1 = {kt: emit_strip_triggers(0, kt, 4, NSTRIP) for kt in range(KT)}
        for kt in range(KT):
            emit_converts(0, kt, 4, st1[kt])
        for kt in range(KT):
            emit_V(0, kt, 1)

        CHUNKS = [(0, 0), (0, 1), (1, 0), (1, 1)]  # (ci, ot)

        for img in range(npc):
            nxt = img + 1
            if nxt < npc:
                for kt in range(KT):
                    emit_pad(nxt, kt)
                strips = {kt: emit_strip_triggers(nxt, kt, 0, NSTRIP)
                          for kt in range(KT)}
            for qi, (ci, ot) in enumerate(CHUNKS):
                emit_chunk(img, ci, ot,
                           last=(img == npc - 1 and qi == len(CHUNKS) - 1))
                if nxt < npc:
                    if qi == 0:
                        for kt in range(KT):
                            emit_converts(nxt, kt, 0, strips[kt][:4])
                    elif qi == 1:
                        for kt in range(KT):
                            emit_V(nxt, kt, 0)
                    elif qi == 2:
                        for kt in range(KT):
                            emit_converts(nxt, kt, 4, strips[kt][4:])
                    else:
                        for kt in range(KT):
                            emit_V(nxt, kt, 1)

    nc.compile()
    return nc


def quantize_weights(w):
    """Match reference fake-quant in f32: returns wq = dequantized weights."""
    w = np.asarray(w, np.float32)
    amax = np.max(np.abs(w), axis=(1, 2, 3), keepdims=True).astype(np.float32)
    scale = np.maximum((amax / np.float32(QMAX)).astype(np.float32),
                       np.float32(SCALING_MIN_VAL)).astype(np.float32)
    q = np.clip(np.rint((w / scale).astype(np.float32)),
                -QMAX, QMAX).astype(np.float32)
    return (q * scale).astype(np.float32)


def pack_weights(wq):
    """wq [O,C,3,3] -> winograd U packed [128, (u,kw,kt,ot,o_loc)] fp16."""
    u6 = np.einsum("ur,ocrk->uock", G_MAT,
                   wq.astype(np.float64)).astype(np.float32)
    a = u6.reshape(6, OT, 128, KT, 128, 3)       # [u, ot, o, kt, c, kw]
    p = a.transpose(4, 0, 5, 3, 1, 2)            # [c, u, kw, kt, ot, o]
    return np.ascontiguousarray(p).reshape(
        128, 6 * 3 * KT * OT * 128).astype(np.float16)


_nc_cache = {}
LAST_RESULT = None  # BassKernelResults of the most recent kernel() call


def kernel(x, w):
    global LAST_RESULT
    x = np.ascontiguousarray(np.asarray(x, np.float32))
    w = np.asarray(w, np.float32)
    assert x.shape == (N, C, H, W) and w.shape == (O, C, KH, KW)

    w_host = pack_weights(quantize_weights(w))

    if "nc" not in _nc_cache:
        _nc_cache["nc"] = build_nc()
    nc = _nc_cache["nc"]

    in_maps = [
        {"x": np.ascontiguousarray(x[cid * NPC:(cid + 1) * NPC]),
         "wu": w_host}
        for cid in range(CORES)
    ]
    kwargs = {}
    trace_dir = os.environ.get("KERNEL_TRACE_DIR")
    if trace_dir:  # dev-harness profiling only; unset in normal use
        kwargs = {"trace": True, "tmpdir": trace_dir}
    res = run_bass_kernel_spmd(nc, in_maps, list(range(CORES)), **kwargs)
    LAST_RESULT = res
    return np.concatenate([res.results[cid]["out"] for cid in range(CORES)],
                          axis=0)


if __name__ == "__main__":
    rng = np.random.default_rng(0)
    x = rng.standard_normal((N, C, H, W), dtype=np.float32)
    w = rng.standard_normal((O, C, KH, KW), dtype=np.float32) * 0.05
    out = kernel(x, w)
    print("out", out.shape, out.dtype, float(np.abs(out).max()))


# revision 8
# speedup vs baseline: 1.1688x; 1.1688x over previous
"""Trainium2 Bass kernel: Brevitas-style int4 fake-quant Conv2d (3x3, pad 1).

reference:
    wq = fake_quant_per_channel(w)          # per-O-channel int4 scale
    out = conv2d(x, wq, NCHW/OIHW, pad 1)

Strategy (v3, 1D Winograd F(2,3) along H):
  * Host: fake-quant w -> wq (f32), fold the Winograd weight transform G
    (and the per-channel scale, already inside wq) into
    U[u, o, c, kw] = sum_r G[u, r] wq[o, c, r, kw], cast fp16.
  * Device (data-parallel, 4 images/core x 8 cores): per image build a
    padded fp16 xp [128, 2kt, 58, 58], then 4 Winograd input planes
    V_u[c, t, col] = B^T[u, :] . xp[c, 2t:2t+4, col] (t = 28 row tiles):
      V0 = r0 - r2   V1 = r1 + r2   V2 = r2 - r1   V3 = r1 - r3
    -- 4 pure tensor_tensor ops (2x_1p DVE mode; scalar_tensor_tensor
    would be 1x-only).  The conv becomes, per (ot, chunk of 7 tiles):
    4 groups M_u [128o, 7, 56] = sum_{kt,kw} U[u,kw,kt,ot].T @ V_u[kt]
    -- 24 matmuls of 392 cols vs 36 direct-conv equivalents: 1.5x fewer
    PE cycles than the direct implicit GEMM.
  * Output transform (A^T = [[1,1,1,0],[0,1,-1,-1]]) during the drain:
      c2=copy(m2) [ACT]  t1=m1+c2, t2=m1-c2 [DVE, 1 PSUM operand each]
      c0=copy(m0), c3=copy(m3) [ACT]  o0=t1+c0, o1=t2-c3 [gpsimd]
    (TT is DVE/Pool-only on trn2; Pool has no PSUM port; TT reads at
    most one PSUM operand -> ACT activation-copies bridge the gap.)
    Output rows interleave 2t+j into the f32 out tile, DMA per chunk.
  * Accuracy: fp16 x/V/U with f32 PSUM — host-sim of this pipeline
    measures absmax rel err ~4e-4 vs the f32 reference (gate 2e-2).
"""

import os
import sys
from contextlib import ExitStack

for _p in ("/opt/trn_rl_repo", "/root/.axon_site/_ro/trn_rl_repo"):
    if os.path.isdir(_p) and _p not in sys.path:
        sys.path.insert(0, _p)

import numpy as np

import concourse.bass as bass  # noqa: F401
import concourse.mybir as mybir
import concourse.tile as tile
from concourse import bacc
from concourse.bass_utils import run_bass_kernel_spmd

F32 = mybir.dt.float32
FP16 = mybir.dt.float16

# Problem shapes (hardcoded per contract).
N, C, H, W = 32, 256, 56, 56
O, KH, KW = 256, 3, 3
CORES = 8
NPC = N // CORES  # images per core

QMAX = 7.0
SCALING_MIN_VAL = 2e-16

KT = C // 128
OT = O // 128
NU = 4                 # winograd taps
TW = H // 2            # 28 winograd row tiles
TR = 7                 # tiles per chunk
NCI = TW // TR         # 4 chunks per (img, ot)
HP, WP = 58, 58        # padded rows/cols
NSTRIP = 7             # x DMA strips of 8 rows
U_ORDER = (1, 2, 0, 3)
PLANE = HP * WP        # fp16 elems per kt plane

# F(2,3): G (weight transform).  B^T/A^T are hardcoded in the op lists.
G_MAT = np.array([
    [1, 0, 0],
    [1 / 2, 1 / 2, 1 / 2],
    [1 / 2, -1 / 2, 1 / 2],
    [0, 0, 1],
], dtype=np.float64)


def build_nc(npc=NPC, warmup_mms=80):
    nc = bacc.Bacc("TRN2", target_bir_lowering=False, debug=False)
    x_d = nc.dram_tensor("x", [npc, C, H, W], F32, kind="ExternalInput").ap()
    w_d = nc.dram_tensor("wu", [128, NU * 3 * KT * OT * 128], FP16,
                         kind="ExternalInput").ap()
    out_d = nc.dram_tensor("out", [npc, O, H, W], F32,
                           kind="ExternalOutput").ap()

    with tile.TileContext(nc) as tc, ExitStack() as ctx:
        wpool = ctx.enter_context(tc.tile_pool(name="wpool", bufs=1))
        xspool = ctx.enter_context(tc.tile_pool(name="xspool", bufs=16))
        hpool = ctx.enter_context(tc.tile_pool(name="hpool", bufs=2))
        vpool = ctx.enter_context(tc.tile_pool(name="vpool", bufs=2))
        s32pool = ctx.enter_context(tc.tile_pool(name="s32pool", bufs=10))
        opool = ctx.enter_context(tc.tile_pool(name="opool", bufs=6))
        ppool = ctx.enter_context(tc.tile_pool(name="ppool", bufs=8,
                                               space="PSUM"))

        w_sb = wpool.tile([128, NU * 3 * KT * OT * 128], FP16)
        nc.scalar.dma_start(w_sb[:, :], w_d[:, :])

        def wslice(u, kw, kt, ot):
            j = (((u * 3 + kw) * KT + kt) * OT + ot) * 128
            return w_sb[:, j:j + 128]

        if warmup_mms:
            wu = wpool.tile([128, 128], FP16)
            nc.vector.memset(wu[:, :], 0.0)
            wu_ps = ppool.tile([128, 512], F32, tag="ps", name="wu_ps")
            for _ in range(warmup_mms):
                nc.tensor.matmul(wu_ps[:, :128], wu[:, :], wu[:, :],
                                 start=True, stop=True)

        xp_t = {}  # img -> xp tile ([128, 2*PLANE] fp16)
        vv = {}    # img -> [128, 2, NU, TW, WP] view

        def emit_pad(img):
            xp = hpool.tile([128, KT * PLANE], FP16, tag="xp")
            xp_t[img] = xp
            for kt in range(KT):
                b = kt * PLANE
                nc.vector.memset(xp[:, b: b + WP + 1], 0.0)
                nc.vector.memset(xp[:, b + (HP - 1) * WP - 1: b + HP * WP],
                                 0.0)
                nc.vector.memset(
                    xp[:, b + 2 * WP - 1: b + 2 * WP - 1 + (HP - 3) * WP]
                    .rearrange("p (a b) -> p a b", b=WP)[:, :, 0:2], 0.0)

        def emit_strip_triggers(img, s0, s1):
            tiles = []
            for kt in range(KT):
                for s in range(s0, s1):
                    xs = xspool.tile([128, 8, W], F32, tag="xs")
                    q = nc.sync if kt == 0 else nc.scalar
                    q.dma_start(xs[:, :, :],
                                x_d[img, kt * 128:(kt + 1) * 128,
                                    8 * s: 8 * s + 8, :])
                    tiles.append((kt, s, xs))
            return tiles

        def emit_converts(img, tiles):
            v = xp_t[img][:, :].rearrange("p (k r c) -> p k r c", k=KT, c=WP)
            for kt, s, xs in tiles:
                nc.scalar.copy(v[:, kt, 1 + 8 * s: 9 + 8 * s, 1:1 + W],
                               xs[:, :, :])

        def emit_V(img, half, per_kt=False):
            """V planes for tiles [half*14, half*14+14)."""
            if img not in vv:
                vt = vpool.tile([128, KT * NU * TW * WP], FP16, tag="v")
                vv[img] = vt[:, :].rearrange(
                    "p (k u t c) -> p k u t c", k=KT, u=NU, c=WP)
            v5 = vv[img]
            ta, tb = half * 14, half * 14 + 14
            xp2 = xp_t[img][:, :].rearrange(
                "p (k t f c) -> p k t f c", k=KT, f=2, c=WP)

            kts = [(kt, kt + 1) for kt in range(KT)] if per_kt \
                else [(0, KT)]
            for ka, kb in kts:
                def R(i):
                    q, rr = divmod(i, 2)
                    return xp2[:, ka:kb, ta + q: tb + q, rr, :]

                def V(u):
                    return v5[:, ka:kb, u, ta:tb, :]

                # emission order matches U_ORDER so img-0 MMs start early
                nc.vector.tensor_add(V(1), R(1), R(2))
                nc.vector.tensor_sub(V(2), R(2), R(1))
                nc.vector.tensor_sub(V(0), R(0), R(2))
                nc.vector.tensor_sub(V(3), R(1), R(3))

        def s32():
            t = s32pool.tile([128, TR * W], F32, tag="s32")
            return t[:, :].rearrange("p (t c) -> p t c", c=W)

        def emit_chunk(img, ci, ot, last=False):
            m = {}
            ob = opool.tile([128, 2 * TR * W], F32, tag="ob")
            ob2 = ob[:, :].rearrange("p (t f c) -> p t f c", f=2, c=W)
            t1 = t2 = None
            for u in U_ORDER:
                ps = ppool.tile([128, 512], F32, tag="ps", name=f"ps{u}")
                mv = ps[:, : TR * W].rearrange("p (t c) -> p t c", c=W)
                m[u] = mv
                idx = 0
                for kt in range(KT):
                    vsl = vv[img][:, kt, u, ci * TR:(ci + 1) * TR, :]
                    for kw in range(3):
                        nc.tensor.matmul(
                            mv[:, :, :], wslice(u, kw, kt, ot),
                            vsl[:, :, kw: kw + W],
                            start=(idx == 0), stop=(idx == 3 * KT - 1),
                        )
                        idx += 1
                # drains trail each accumulation group. TT is DVE/Pool-only,
                # Pool can't read PSUM, TT reads <=1 PSUM operand, so ACT
                # activation-copies bridge PSUM->SBUF for the second inputs.
                if u == 2:
                    c2 = s32()
                    nc.scalar.copy(c2, m[2])
                    t1, t2 = s32(), s32()
                    nc.vector.tensor_add(t1, m[1], c2)   # m1 + m2
                    nc.vector.tensor_sub(t2, m[1], c2)   # m1 - m2
                if u == 0:
                    c0 = s32()
                    nc.scalar.copy(c0, m[0])
                    nc.gpsimd.tensor_add(ob2[:, :, 0, :], t1, c0)
            c3 = s32()
            nc.scalar.copy(c3, m[3])
            nc.gpsimd.tensor_sub(ob2[:, :, 1, :], t2, c3)

            dst = out_d[img, ot * 128:(ot + 1) * 128,
                        2 * TR * ci: 2 * TR * (ci + 1), :]
            obv = ob[:, :].rearrange("p (r c) -> p r c", c=W)
            if last:
                for a0, b0 in ((0, TR), (TR, 2 * TR)):
                    nc.sync.dma_start(dst[:, a0:b0, :], obv[:, a0:b0, :])
            else:
                nc.sync.dma_start(dst[:, :, :], obv[:, :, :])

        # ---------------- schedule ----------------
        # image 0: paced by DMA; V per-kt halves so first MMs start early
        emit_pad(0)
        st0 = emit_strip_triggers(0, 0, 4)
        emit_converts(0, st0)
        emit_V(0, 0, per_kt=True)
        st1 = emit_strip_triggers(0, 4, NSTRIP)
        emit_converts(0, st1)
        emit_V(0, 1)

        CHUNKS = [(ci, ot) for ci in range(NCI) for ot in range(OT)]

        for img in range(npc):
            nxt = img + 1
            if nxt < npc:
                emit_pad(nxt)
                strips = emit_strip_triggers(nxt, 0, NSTRIP)
            for qi, (ci, ot) in enumerate(CHUNKS):
                emit_chunk(img, ci, ot,
                           last=(img == npc - 1 and qi == len(CHUNKS) - 1))
                if nxt < npc:
                    if qi == 0:
                        emit_converts(nxt, [t for t in strips if t[1] < 4])
                    elif qi == 2:
                        emit_V(nxt, 0)
                    elif qi == 4:
                        emit_converts(nxt, [t for t in strips if t[1] >= 4])
                    elif qi == 6:
                        emit_V(nxt, 1)

    nc.compile()
    return nc


def quantize_weights(w):
    """Match reference fake-quant in f32: returns wq = dequantized weights."""
    w = np.asarray(w, np.float32)
    amax = np.max(np.abs(w), axis=(1, 2, 3), keepdims=True).astype(np.float32)
    scale = np.maximum((amax / np.float32(QMAX)).astype(np.float32),
                       np.float32(SCALING_MIN_VAL)).astype(np.float32)
    q = np.clip(np.rint((w / scale).astype(np.float32)),
                -QMAX, QMAX).astype(np.float32)
    return (q * scale).astype(np.float32)


def pack_weights(wq):
    """wq [O,C,3,3] -> winograd U packed [128, (u,kw,kt,ot,o_loc)] fp16."""
    u4 = np.einsum("ur,ocrk->uock", G_MAT,
                   wq.astype(np.float64)).astype(np.float32)
    a = u4.reshape(NU, OT, 128, KT, 128, 3)      # [u, ot, o, kt, c, kw]
    p = a.transpose(4, 0, 5, 3, 1, 2)            # [c, u, kw, kt, ot, o]
    return np.ascontiguousarray(p).reshape(
        128, NU * 3 * KT * OT * 128).astype(np.float16)


_nc_cache = {}
LAST_RESULT = None  # BassKernelResults of the most recent kernel() call


def kernel(x, w):
    global LAST_RESULT
    x = np.ascontiguousarray(np.asarray(x, np.float32))
    w = np.asarray(w, np.float32)
    assert x.shape == (N, C, H, W) and w.shape == (O, C, KH, KW)

    w_host = pack_weights(quantize_weights(w))

    if "nc" not in _nc_cache:
        _nc_cache["nc"] = build_nc()
    nc = _nc_cache["nc"]

    in_maps = [
        {"x": np.ascontiguousarray(x[cid * NPC:(cid + 1) * NPC]),
         "wu": w_host}
        for cid in range(CORES)
    ]
    kwargs = {}
    trace_dir = os.environ.get("KERNEL_TRACE_DIR")
    if trace_dir:  # dev-harness profiling only; unset in normal use
        kwargs = {"trace": True, "tmpdir": trace_dir}
    res = run_bass_kernel_spmd(nc, in_maps, list(range(CORES)), **kwargs)
    LAST_RESULT = res
    return np.concatenate([res.results[cid]["out"] for cid in range(CORES)],
                          axis=0)


if __name__ == "__main__":
    rng = np.random.default_rng(0)
    x = rng.standard_normal((N, C, H, W), dtype=np.float32)
    w = rng.standard_normal((O, C, KH, KW), dtype=np.float32) * 0.05
    out = kernel(x, w)
    print("out", out.shape, out.dtype, float(np.abs(out).max()))


# revision 10
# speedup vs baseline: 1.3735x; 1.1752x over previous
"""Trainium2 Bass kernel: Brevitas-style int4 fake-quant Conv2d (3x3, pad 1).

reference:
    wq = fake_quant_per_channel(w)          # per-O-channel int4 scale
    out = conv2d(x, wq, NCHW/OIHW, pad 1)

Strategy (v3, 1D Winograd F(2,3) along H):
  * Host: fake-quant w -> wq (f32), fold the Winograd weight transform G
    (and the per-channel scale, already inside wq) into
    U[u, o, c, kw] = sum_r G[u, r] wq[o, c, r, kw], cast fp16.
  * Device (data-parallel, 4 images/core x 8 cores): per image build a
    padded fp16 xp [128, 2kt, 58, 58], then 4 Winograd input planes
    V_u[c, t, col] = B^T[u, :] . xp[c, 2t:2t+4, col] (t = 28 row tiles):
      V0 = r0 - r2   V1 = r1 + r2   V2 = r2 - r1   V3 = r1 - r3
    -- 4 pure tensor_tensor ops (2x_1p DVE mode; scalar_tensor_tensor
    would be 1x-only).  The conv becomes, per (ot, chunk of 7 tiles):
    4 groups M_u [128o, 7, 56] = sum_{kt,kw} U[u,kw,kt,ot].T @ V_u[kt]
    -- 24 matmuls of 392 cols vs 36 direct-conv equivalents: 1.5x fewer
    PE cycles than the direct implicit GEMM.
  * Output transform (A^T = [[1,1,1,0],[0,1,-1,-1]]) during the drain:
      c2=copy(m2) [ACT]  t1=m1+c2, t2=m1-c2 [DVE, 1 PSUM operand each]
      c0=copy(m0), c3=copy(m3) [ACT]  o0=t1+c0, o1=t2-c3 [gpsimd]
    (TT is DVE/Pool-only on trn2; Pool has no PSUM port; TT reads at
    most one PSUM operand -> ACT activation-copies bridge the gap.)
    Output rows interleave 2t+j into the f32 out tile, DMA per chunk.
  * Accuracy: fp16 x/V/U with f32 PSUM — host-sim of this pipeline
    measures absmax rel err ~4e-4 vs the f32 reference (gate 2e-2).
"""

import os
import sys
from contextlib import ExitStack

for _p in ("/opt/trn_rl_repo", "/root/.axon_site/_ro/trn_rl_repo"):
    if os.path.isdir(_p) and _p not in sys.path:
        sys.path.insert(0, _p)

import numpy as np

import concourse.bass as bass  # noqa: F401
import concourse.mybir as mybir
import concourse.tile as tile
from concourse import bacc
from concourse.bass_utils import run_bass_kernel_spmd

F32 = mybir.dt.float32
FP16 = mybir.dt.float16

# Problem shapes (hardcoded per contract).
N, C, H, W = 32, 256, 56, 56
O, KH, KW = 256, 3, 3
CORES = 8
NPC = N // CORES  # images per core

QMAX = 7.0
SCALING_MIN_VAL = 2e-16

KT = C // 128
OT = O // 128
NU = 4                 # winograd taps
TW = H // 2            # 28 winograd row tiles
TR = 7                 # tiles per chunk
NCI = TW // TR         # 4 chunks per (img, ot)
HP, WP = 58, 58        # padded rows/cols
NSTRIP = 7             # x DMA strips of 8 rows
U_ORDER = (1, 2, 0, 3)
PLANE = HP * WP        # fp16 elems per kt plane

# F(2,3): G (weight transform).  B^T/A^T are hardcoded in the op lists.
G_MAT = np.array([
    [1, 0, 0],
    [1 / 2, 1 / 2, 1 / 2],
    [1 / 2, -1 / 2, 1 / 2],
    [0, 0, 1],
], dtype=np.float64)


def build_nc(npc=NPC, warmup_mms=80):
    nc = bacc.Bacc("TRN2", target_bir_lowering=False, debug=False)
    x_d = nc.dram_tensor("x", [npc, C, H, W], F32, kind="ExternalInput").ap()
    w_d = nc.dram_tensor("wu", [128, NU * 3 * KT * OT * 128], FP16,
                         kind="ExternalInput").ap()
    out_d = nc.dram_tensor("out", [npc, O, H, W], F32,
                           kind="ExternalOutput").ap()

    with tile.TileContext(nc) as tc, ExitStack() as ctx:
        wpool = ctx.enter_context(tc.tile_pool(name="wpool", bufs=1))
        xspool = ctx.enter_context(tc.tile_pool(name="xspool", bufs=16))
        hpool = ctx.enter_context(tc.tile_pool(name="hpool", bufs=2))
        vpool = ctx.enter_context(tc.tile_pool(name="vpool", bufs=2))
        s32pool = ctx.enter_context(tc.tile_pool(name="s32pool", bufs=10))
        opool = ctx.enter_context(tc.tile_pool(name="opool", bufs=6))
        ppool = ctx.enter_context(tc.tile_pool(name="ppool", bufs=8,
                                               space="PSUM"))

        w_sb = wpool.tile([128, NU * 3 * KT * OT * 128], FP16)
        nc.scalar.dma_start(w_sb[:, :], w_d[:, :])

        def wslice(u, kw, kt, ot):
            j = (((u * 3 + kw) * KT + kt) * OT + ot) * 128
            return w_sb[:, j:j + 128]

        if warmup_mms:
            # steady-state-shaped warmup: FD=392 MMs cycling distinct
            # weight slices (LDW per MM, like the real stream)
            wu = wpool.tile([128, 392], FP16)
            nc.vector.memset(wu[:, :], 0.0)
            wu_ps = ppool.tile([128, 512], F32, tag="ps", name="wu_ps")
            for i in range(warmup_mms):
                nc.tensor.matmul(wu_ps[:, :392], wu[:, :128],
                                 wu[:, :].rearrange(
                                     "p (t c) -> p t c", c=56),
                                 start=True, stop=True)

        xp_t = {}  # img -> xp tile ([128, 2*PLANE] fp16)
        vv = {}    # img -> [128, 2, NU, TW, WP] view

        def emit_pad(img):
            xp = hpool.tile([128, KT * PLANE], FP16, tag="xp")
            xp_t[img] = xp
            for kt in range(KT):
                b = kt * PLANE
                nc.vector.memset(xp[:, b: b + WP + 1], 0.0)
                nc.vector.memset(xp[:, b + (HP - 1) * WP - 1: b + HP * WP],
                                 0.0)
                nc.vector.memset(
                    xp[:, b + 2 * WP - 1: b + 2 * WP - 1 + (HP - 3) * WP]
                    .rearrange("p (a b) -> p a b", b=WP)[:, :, 0:2], 0.0)

        def emit_strip_triggers(img, s0, s1):
            tiles = []
            for kt in range(KT):
                for s in range(s0, s1):
                    xs = xspool.tile([128, 8, W], F32, tag="xs")
                    q = nc.sync if kt == 0 else nc.scalar
                    q.dma_start(xs[:, :, :],
                                x_d[img, kt * 128:(kt + 1) * 128,
                                    8 * s: 8 * s + 8, :])
                    tiles.append((kt, s, xs))
            return tiles

        def emit_converts(img, tiles):
            v = xp_t[img][:, :].rearrange("p (k r c) -> p k r c", k=KT, c=WP)
            for kt, s, xs in tiles:
                nc.scalar.copy(v[:, kt, 1 + 8 * s: 9 + 8 * s, 1:1 + W],
                               xs[:, :, :])

        def emit_V(img, half, per_kt=False):
            """V planes for tiles [half*14, half*14+14)."""
            if img not in vv:
                vt = vpool.tile([128, KT * NU * TW * WP], FP16, tag="v")
                vv[img] = vt[:, :].rearrange(
                    "p (k u t c) -> p k u t c", k=KT, u=NU, c=WP)
            v5 = vv[img]
            ta, tb = half * 14, half * 14 + 14
            xp2 = xp_t[img][:, :].rearrange(
                "p (k t f c) -> p k t f c", k=KT, f=2, c=WP)

            kts = [(kt, kt + 1) for kt in range(KT)] if per_kt \
                else [(0, KT)]
            for ka, kb in kts:
                def R(i):
                    q, rr = divmod(i, 2)
                    return xp2[:, ka:kb, ta + q: tb + q, rr, :]

                def V(u):
                    return v5[:, ka:kb, u, ta:tb, :]

                # emission order matches U_ORDER so img-0 MMs start early
                nc.vector.tensor_add(V(1), R(1), R(2))
                nc.vector.tensor_sub(V(2), R(2), R(1))
                nc.vector.tensor_sub(V(0), R(0), R(2))
                nc.vector.tensor_sub(V(3), R(1), R(3))

        def s32():
            t = s32pool.tile([128, TR * W], F32, tag="s32")
            return t[:, :].rearrange("p (t c) -> p t c", c=W)

        def emit_chunk(img, ci, ot, last=False):
            m = {}
            ob = opool.tile([128, 2 * TR * W], F32, tag="ob")
            ob2 = ob[:, :].rearrange("p (t f c) -> p t f c", f=2, c=W)
            t1 = t2 = None
            for u in U_ORDER:
                ps = ppool.tile([128, 512], F32, tag="ps", name=f"ps{u}")
                mv = ps[:, : TR * W].rearrange("p (t c) -> p t c", c=W)
                m[u] = mv
                idx = 0
                for kt in range(KT):
                    vsl = vv[img][:, kt, u, ci * TR:(ci + 1) * TR, :]
                    for kw in range(3):
                        nc.tensor.matmul(
                            mv[:, :, :], wslice(u, kw, kt, ot),
                            vsl[:, :, kw: kw + W],
                            start=(idx == 0), stop=(idx == 3 * KT - 1),
                        )
                        idx += 1
                # drains trail each accumulation group. TT is DVE/Pool-only,
                # Pool can't read PSUM, TT reads <=1 PSUM operand, so ACT
                # activation-copies bridge PSUM->SBUF for the second inputs.
                if u == 2:
                    c2 = s32()
                    nc.scalar.copy(c2, m[2])
                    t1, t2 = s32(), s32()
                    nc.vector.tensor_add(t1, m[1], c2)   # m1 + m2
                    nc.vector.tensor_sub(t2, m[1], c2)   # m1 - m2
                if u == 0:
                    c0 = s32()
                    nc.scalar.copy(c0, m[0])
                    oeng = nc.vector if img == npc - 1 else nc.gpsimd
                    oeng.tensor_add(ob2[:, :, 0, :], t1, c0)
            c3 = s32()
            nc.scalar.copy(c3, m[3])
            oeng = nc.vector if img == npc - 1 else nc.gpsimd
            oeng.tensor_sub(ob2[:, :, 1, :], t2, c3)

            dst = out_d[img, ot * 128:(ot + 1) * 128,
                        2 * TR * ci: 2 * TR * (ci + 1), :]
            obv = ob[:, :].rearrange("p (r c) -> p r c", c=W)
            if last:
                for a0, b0 in ((0, TR), (TR, 2 * TR)):
                    nc.sync.dma_start(dst[:, a0:b0, :], obv[:, a0:b0, :])
            else:
                nc.sync.dma_start(dst[:, :, :], obv[:, :, :])

        # ---------------- schedule ----------------
        # image 0: paced by DMA; V per-kt halves so first MMs start early
        emit_pad(0)
        st0 = emit_strip_triggers(0, 0, 4)
        emit_converts(0, st0)
        emit_V(0, 0, per_kt=True)
        st1 = emit_strip_triggers(0, 4, NSTRIP)
        emit_converts(0, st1)
        emit_V(0, 1)

        CHUNKS = [(ci, ot) for ci in range(NCI) for ot in range(OT)]

        for img in range(npc):
            nxt = img + 1
            if nxt < npc:
                emit_pad(nxt)
                strips = emit_strip_triggers(nxt, 0, NSTRIP)
            for qi, (ci, ot) in enumerate(CHUNKS):
                emit_chunk(img, ci, ot,
                           last=(img == npc - 1 and qi == len(CHUNKS) - 1))
                if nxt < npc:
                    if qi == 0:
                        emit_converts(nxt, [t for t in strips if t[1] < 4])
                    elif qi == 2:
                        emit_V(nxt, 0)
                    elif qi == 4:
                        emit_converts(nxt, [t for t in strips if t[1] >= 4])
                    elif qi == 6:
                        emit_V(nxt, 1)

    nc.compile()
    return nc


def quantize_weights(w):
    """Match reference fake-quant in f32: returns wq = dequantized weights."""
    w = np.asarray(w, np.float32)
    amax = np.max(np.abs(w), axis=(1, 2, 3), keepdims=True).astype(np.float32)
    scale = np.maximum((amax / np.float32(QMAX)).astype(np.float32),
                       np.float32(SCALING_MIN_VAL)).astype(np.float32)
    q = np.clip(np.rint((w / scale).astype(np.float32)),
                -QMAX, QMAX).astype(np.float32)
    return (q * scale).astype(np.float32)


def pack_weights(wq):
    """wq [O,C,3,3] -> winograd U packed [128, (u,kw,kt,ot,o_loc)] fp16."""
    u4 = np.einsum("ur,ocrk->uock", G_MAT,
                   wq.astype(np.float64)).astype(np.float32)
    a = u4.reshape(NU, OT, 128, KT, 128, 3)      # [u, ot, o, kt, c, kw]
    p = a.transpose(4, 0, 5, 3, 1, 2)            # [c, u, kw, kt, ot, o]
    return np.ascontiguousarray(p).reshape(
        128, NU * 3 * KT * OT * 128).astype(np.float16)


_nc_cache = {}
LAST_RESULT = None  # BassKernelResults of the most recent kernel() call


def kernel(x, w):
    global LAST_RESULT
    x = np.ascontiguousarray(np.asarray(x, np.float32))
    w = np.asarray(w, np.float32)
    assert x.shape == (N, C, H, W) and w.shape == (O, C, KH, KW)

    w_host = pack_weights(quantize_weights(w))

    if "nc" not in _nc_cache:
        _nc_cache["nc"] = build_nc()
    nc = _nc_cache["nc"]

    in_maps = [
        {"x": np.ascontiguousarray(x[cid * NPC:(cid + 1) * NPC]),
         "wu": w_host}
        for cid in range(CORES)
    ]
    kwargs = {}
    trace_dir = os.environ.get("KERNEL_TRACE_DIR")
    if trace_dir:  # dev-harness profiling only; unset in normal use
        kwargs = {"trace": True, "tmpdir": trace_dir}
    res = run_bass_kernel_spmd(nc, in_maps, list(range(CORES)), **kwargs)
    LAST_RESULT = res
    return np.concatenate([res.results[cid]["out"] for cid in range(CORES)],
                          axis=0)


if __name__ == "__main__":
    rng = np.random.default_rng(0)
    x = rng.standard_normal((N, C, H, W), dtype=np.float32)
    w = rng.standard_normal((O, C, KH, KW), dtype=np.float32) * 0.05
    out = kernel(x, w)
    print("out", out.shape, out.dtype, float(np.abs(out).max()))


# revision 14
# speedup vs baseline: 1.3795x; 1.0044x over previous
"""Trainium2 Bass kernel: Brevitas-style int4 fake-quant Conv2d (3x3, pad 1).

reference:
    wq = fake_quant_per_channel(w)          # per-O-channel int4 scale
    out = conv2d(x, wq, NCHW/OIHW, pad 1)

Strategy (v3, 1D Winograd F(2,3) along H):
  * Host: fake-quant w -> wq (f32), fold the Winograd weight transform G
    (and the per-channel scale, already inside wq) into
    U[u, o, c, kw] = sum_r G[u, r] wq[o, c, r, kw], cast fp16.
  * Device (data-parallel, 4 images/core x 8 cores): per image build a
    padded fp16 xp [128, 2kt, 58, 58], then 4 Winograd input planes
    V_u[c, t, col] = B^T[u, :] . xp[c, 2t:2t+4, col] (t = 28 row tiles):
      V0 = r0 - r2   V1 = r1 + r2   V2 = r2 - r1   V3 = r1 - r3
    -- 4 pure tensor_tensor ops (2x_1p DVE mode; scalar_tensor_tensor
    would be 1x-only).  The conv becomes, per (ot, chunk of 7 tiles):
    4 groups M_u [128o, 7, 56] = sum_{kt,kw} U[u,kw,kt,ot].T @ V_u[kt]
    -- 24 matmuls of 392 cols vs 36 direct-conv equivalents: 1.5x fewer
    PE cycles than the direct implicit GEMM.
  * Output transform (A^T = [[1,1,1,0],[0,1,-1,-1]]) during the drain:
      c2=copy(m2) [ACT]  t1=m1+c2, t2=m1-c2 [DVE, 1 PSUM operand each]
      c0=copy(m0), c3=copy(m3) [ACT]  o0=t1+c0, o1=t2-c3 [gpsimd]
    (TT is DVE/Pool-only on trn2; Pool has no PSUM port; TT reads at
    most one PSUM operand -> ACT activation-copies bridge the gap.)
    Output rows interleave 2t+j into the f32 out tile, DMA per chunk.
  * Accuracy: fp16 x/V/U with f32 PSUM — host-sim of this pipeline
    measures absmax rel err ~4e-4 vs the f32 reference (gate 2e-2).
"""

import os
import sys
from contextlib import ExitStack

for _p in ("/opt/trn_rl_repo", "/root/.axon_site/_ro/trn_rl_repo"):
    if os.path.isdir(_p) and _p not in sys.path:
        sys.path.insert(0, _p)

import numpy as np

import concourse.bass as bass  # noqa: F401
import concourse.mybir as mybir
import concourse.tile as tile
from concourse import bacc
from concourse.bass_utils import run_bass_kernel_spmd

F32 = mybir.dt.float32
FP16 = mybir.dt.float16

# Problem shapes (hardcoded per contract).
N, C, H, W = 32, 256, 56, 56
O, KH, KW = 256, 3, 3
CORES = 8
NPC = N // CORES  # images per core

QMAX = 7.0
SCALING_MIN_VAL = 2e-16

KT = C // 128
OT = O // 128
NU = 4                 # winograd taps
TW = H // 2            # 28 winograd row tiles
TR = 7                 # tiles per chunk
NCI = TW // TR         # 4 chunks per (img, ot)
HP, WP = 58, 58        # padded rows/cols
NSTRIP = 7             # x DMA strips of 8 rows
U_ORDER = (1, 2, 0, 3)
PLANE = HP * WP        # fp16 elems per kt plane

# F(2,3): G (weight transform).  B^T/A^T are hardcoded in the op lists.
G_MAT = np.array([
    [1, 0, 0],
    [1 / 2, 1 / 2, 1 / 2],
    [1 / 2, -1 / 2, 1 / 2],
    [0, 0, 1],
], dtype=np.float64)


def build_nc(npc=NPC, warmup_mms=45):
    nc = bacc.Bacc("TRN2", target_bir_lowering=False, debug=False)
    x_d = nc.dram_tensor("x", [npc, C, H, W], F32, kind="ExternalInput").ap()
    w_d = nc.dram_tensor("wu", [128, NU * 3 * KT * OT * 128], FP16,
                         kind="ExternalInput").ap()
    out_d = nc.dram_tensor("out", [npc, O, H, W], F32,
                           kind="ExternalOutput").ap()

    with tile.TileContext(nc) as tc, ExitStack() as ctx:
        wpool = ctx.enter_context(tc.tile_pool(name="wpool", bufs=1))
        xspool = ctx.enter_context(tc.tile_pool(name="xspool", bufs=16))
        hpool = ctx.enter_context(tc.tile_pool(name="hpool", bufs=2))
        vpool = ctx.enter_context(tc.tile_pool(name="vpool", bufs=2))
        s32pool = ctx.enter_context(tc.tile_pool(name="s32pool", bufs=10))
        opool = ctx.enter_context(tc.tile_pool(name="opool", bufs=6))
        ppool = ctx.enter_context(tc.tile_pool(name="ppool", bufs=8,
                                               space="PSUM"))

        w_sb = wpool.tile([128, NU * 3 * KT * OT * 128], FP16)
        nc.scalar.dma_start(w_sb[:, :], w_d[:, :])

        def wslice(u, kw, kt, ot):
            j = (((u * 3 + kw) * KT + kt) * OT + ot) * 128
            return w_sb[:, j:j + 128]

        if warmup_mms:
            # steady-state-shaped warmup: FD=392 MMs (LDW per MM, like the
            # real stream); memset on gpsimd, whose program loads earliest
            wu = wpool.tile([128, 392], FP16)
            nc.gpsimd.memset(wu[:, :], 0.0)
            wu_ps = ppool.tile([128, 512], F32, tag="ps", name="wu_ps")
            for i in range(warmup_mms):
                nc.tensor.matmul(wu_ps[:, :392], wu[:, :128],
                                 wu[:, :].rearrange(
                                     "p (t c) -> p t c", c=56),
                                 start=True, stop=True)

        xp_t = {}  # img -> xp tile ([128, 2*PLANE] fp16)
        vv = {}    # img -> [128, 2, NU, TW, WP] view

        def emit_pad(img):
            xp = hpool.tile([128, KT * PLANE], FP16, tag="xp")
            xp_t[img] = xp
            for kt in range(KT):
                b = kt * PLANE
                nc.vector.memset(xp[:, b: b + WP + 1], 0.0)
                nc.vector.memset(xp[:, b + (HP - 1) * WP - 1: b + HP * WP],
                                 0.0)
                nc.vector.memset(
                    xp[:, b + 2 * WP - 1: b + 2 * WP - 1 + (HP - 3) * WP]
                    .rearrange("p (a b) -> p a b", b=WP)[:, :, 0:2], 0.0)

        def emit_strip_triggers(img, s0, s1):
            tiles = []
            for kt in range(KT):
                for s in range(s0, s1):
                    xs = xspool.tile([128, 8, W], F32, tag="xs")
                    q = nc.sync if kt == 0 else nc.scalar
                    q.dma_start(xs[:, :, :],
                                x_d[img, kt * 128:(kt + 1) * 128,
                                    8 * s: 8 * s + 8, :])
                    tiles.append((kt, s, xs))
            return tiles

        def emit_converts(img, tiles):
            v = xp_t[img][:, :].rearrange("p (k r c) -> p k r c", k=KT, c=WP)
            for kt, s, xs in tiles:
                nc.scalar.copy(v[:, kt, 1 + 8 * s: 9 + 8 * s, 1:1 + W],
                               xs[:, :, :])

        def emit_V(img, half, per_kt=False):
            """V planes for tiles [half*14, half*14+14)."""
            if img not in vv:
                vt = vpool.tile([128, KT * NU * TW * WP], FP16, tag="v")
                vv[img] = vt[:, :].rearrange(
                    "p (k u t c) -> p k u t c", k=KT, u=NU, c=WP)
            v5 = vv[img]
            ta, tb = half * 14, half * 14 + 14
            xp2 = xp_t[img][:, :].rearrange(
                "p (k t f c) -> p k t f c", k=KT, f=2, c=WP)

            kts = [(kt, kt + 1) for kt in range(KT)] if per_kt \
                else [(0, KT)]

            def R(ka, kb, i):
                q, rr = divmod(i, 2)
                return xp2[:, ka:kb, ta + q: tb + q, rr, :]

            def V(ka, kb, u):
                return v5[:, ka:kb, u, ta:tb, :]

            # op-major across kt, in U_ORDER, so img-0 MM groups (which
            # consume both kt planes of one u) start as early as possible
            for op, u in ((nc.vector.tensor_add, 1),
                          (nc.vector.tensor_sub, 2),
                          (nc.vector.tensor_sub, 0),
                          (nc.vector.tensor_sub, 3)):
                ins = {1: (1, 2), 2: (2, 1), 0: (0, 2), 3: (1, 3)}[u]
                for ka, kb in kts:
                    op(V(ka, kb, u), R(ka, kb, ins[0]), R(ka, kb, ins[1]))

        def s32():
            t = s32pool.tile([128, TR * W], F32, tag="s32")
            return t[:, :].rearrange("p (t c) -> p t c", c=W)

        def emit_chunk(img, ci, ot, last=False):
            m = {}
            ob = opool.tile([128, 2 * TR * W], F32, tag="ob")
            ob2 = ob[:, :].rearrange("p (t f c) -> p t f c", f=2, c=W)
            t1 = t2 = None
            for u in U_ORDER:
                ps = ppool.tile([128, 512], F32, tag="ps", name=f"ps{u}")
                mv = ps[:, : TR * W].rearrange("p (t c) -> p t c", c=W)
                m[u] = mv
                idx = 0
                for kt in range(KT):
                    vsl = vv[img][:, kt, u, ci * TR:(ci + 1) * TR, :]
                    for kw in range(3):
                        nc.tensor.matmul(
                            mv[:, :, :], wslice(u, kw, kt, ot),
                            vsl[:, :, kw: kw + W],
                            start=(idx == 0), stop=(idx == 3 * KT - 1),
                        )
                        idx += 1
                # drains trail each accumulation group. TT is DVE/Pool-only,
                # Pool can't read PSUM, TT reads <=1 PSUM operand, so ACT
                # activation-copies bridge PSUM->SBUF for the second inputs.
                if u == 2:
                    c2 = s32()
                    nc.scalar.copy(c2, m[2])
                    t1, t2 = s32(), s32()
                    nc.vector.tensor_add(t1, m[1], c2)   # m1 + m2
                    nc.vector.tensor_sub(t2, m[1], c2)   # m1 - m2
                if u == 0:
                    c0 = s32()
                    nc.scalar.copy(c0, m[0])
                    oeng = nc.vector if img == npc - 1 else nc.gpsimd
                    oeng.tensor_add(ob2[:, :, 0, :], t1, c0)
            c3 = s32()
            nc.scalar.copy(c3, m[3])
            oeng = nc.vector if img == npc - 1 else nc.gpsimd
            oeng.tensor_sub(ob2[:, :, 1, :], t2, c3)

            dst = out_d[img, ot * 128:(ot + 1) * 128,
                        2 * TR * ci: 2 * TR * (ci + 1), :]
            obv = ob[:, :].rearrange("p (r c) -> p r c", c=W)
            if last:
                for a0, b0 in ((0, TR), (TR, 2 * TR)):
                    nc.sync.dma_start(dst[:, a0:b0, :], obv[:, a0:b0, :])
            else:
                nc.sync.dma_start(dst[:, :, :], obv[:, :, :])

        # ---------------- schedule ----------------
        # image 0: paced by DMA; V per-kt halves so first MMs start early.
        # V half-1 is emitted between the first chunks (not before them) so
        # chunk-0 drains aren't queued behind it on the DVE.
        emit_pad(0)
        st0 = emit_strip_triggers(0, 0, 4)
        emit_converts(0, st0)
        emit_V(0, 0, per_kt=True)
        st1 = emit_strip_triggers(0, 4, NSTRIP)

        CHUNKS = [(ci, ot) for ci in range(NCI) for ot in range(OT)]

        for img in range(npc):
            nxt = img + 1
            if nxt < npc:
                emit_pad(nxt)
                strips = emit_strip_triggers(nxt, 0, NSTRIP)
            for qi, (ci, ot) in enumerate(CHUNKS):
                emit_chunk(img, ci, ot,
                           last=(img == npc - 1 and qi == len(CHUNKS) - 1))
                if img == 0 and qi == 0:
                    emit_converts(0, st1)
                if img == 0 and qi == 1:
                    emit_V(0, 1)
                if nxt < npc:
                    if qi == 0:
                        emit_converts(nxt, [t for t in strips if t[1] < 4])
                    elif qi == 2:
                        emit_V(nxt, 0)
                    elif qi == 4:
                        emit_converts(nxt, [t for t in strips if t[1] >= 4])
                    elif qi == 6:
                        emit_V(nxt, 1)

    nc.compile()
    return nc


def quantize_weights(w):
    """Match reference fake-quant in f32: returns wq = dequantized weights."""
    w = np.asarray(w, np.float32)
    amax = np.max(np.abs(w), axis=(1, 2, 3), keepdims=True).astype(np.float32)
    scale = np.maximum((amax / np.float32(QMAX)).astype(np.float32),
                       np.float32(SCALING_MIN_VAL)).astype(np.float32)
    q = np.clip(np.rint((w / scale).astype(np.float32)),
                -QMAX, QMAX).astype(np.float32)
    return (q * scale).astype(np.float32)


def pack_weights(wq):
    """wq [O,C,3,3] -> winograd U packed [128, (u,kw,kt,ot,o_loc)] fp16."""
    u4 = np.einsum("ur,ocrk->uock", G_MAT,
                   wq.astype(np.float64)).astype(np.float32)
    a = u4.reshape(NU, OT, 128, KT, 128, 3)      # [u, ot, o, kt, c, kw]
    p = a.transpose(4, 0, 5, 3, 1, 2)            # [c, u, kw, kt, ot, o]
    return np.ascontiguousarray(p).reshape(
        128, NU * 3 * KT * OT * 128).astype(np.float16)


_nc_cache = {}
LAST_RESULT = None  # BassKernelResults of the most recent kernel() call


def kernel(x, w):
    global LAST_RESULT
    x = np.ascontiguousarray(np.asarray(x, np.float32))
    w = np.asarray(w, np.float32)
    assert x.shape == (N, C, H, W) and w.shape == (O, C, KH, KW)

    w_host = pack_weights(quantize_weights(w))

    if "nc" not in _nc_cache:
        _nc_cache["nc"] = build_nc()
    nc = _nc_cache["nc"]

    in_maps = [
        {"x": np.ascontiguousarray(x[cid * NPC:(cid + 1) * NPC]),
         "wu": w_host}
        for cid in range(CORES)
    ]
    kwargs = {}
    trace_dir = os.environ.get("KERNEL_TRACE_DIR")
    if trace_dir:  # dev-harness profiling only; unset in normal use
        kwargs = {"trace": True, "tmpdir": trace_dir}
    res = run_bass_kernel_spmd(nc, in_maps, list(range(CORES)), **kwargs)
    LAST_RESULT = res
    return np.concatenate([res.results[cid]["out"] for cid in range(CORES)],
                          axis=0)


if __name__ == "__main__":
    rng = np.random.default_rng(0)
    x = rng.standard_normal((N, C, H, W), dtype=np.float32)
    w = rng.standard_normal((O, C, KH, KW), dtype=np.float32) * 0.05
    out = kernel(x, w)
    print("out", out.shape, out.dtype, float(np.abs(out).max()))
